# revision 3
# baseline (speedup 1.0000x reference)
"""Trainium2 Bass kernel for nn_CounterexampleGenerator (dense_mlp, memory-bound).

Strategy (8 NeuronCores, SPMD, no collectives):
  - Data-parallel over batch B=64: each core gets 8 batches = a contiguous
    [8192, 2048] f32 slice of x (64 MiB) — reading it once is the roofline.
  - Phase A (per core): stream x in 2 MiB tiles [128, 2, 2048]; one DVE add
    folds the two 128-row groups; PE column-sum matmuls (lhsT = tile chunk,
    rhs = ones) accumulate the L-reduction into PSUM, producing pooled in a
    TRANSPOSED layout pooledT[p, dc, b] = pooled[b, dc*128+p] (d on
    partitions) so the PGD loop needs no transposes at all.
  - Phase B: 10 PGD steps on [128, 16, 8] tiles. z1T = sum_dc W1c.T @ xaTc
    (W1 chunks stationary); u = W2 * gelu'(z1T + b1) via one ScalarE
    Derivative_Gelu (b1 is a per-partition bias in this layout) and one
    per-partition tensor_scalar_mul; gT chunks = W1Tc.T @ u (host-supplied
    W1.T chunks stationary); sign + clip updates on VectorE. The positive
    sigmoid' factor is dropped — it cannot change sign(grad).
  - Final score: Gelu, W2 matmul, Sigmoid; each core emits worst_score[1,8].
  - Host: gather 8x8 scores, cert = 1 - ws, violated = min(ws) < 0.1,
    x passes through untouched.
"""

import numpy as np

B, L, D, H = 64, 1024, 2048, 128
N_CORES = 8
BPC = B // N_CORES  # batches per core = 8
DC = D // 128  # 16 d-chunks
BUDGET = 10
STEP = 0.01
EPS = 0.2

# Phase-A tiling: each DMA tile holds LC row-groups of 128 rows x D cols.
LC = 2  # row-groups per tile -> [128, LC*2048] f32 = 2 MiB
TILES_PER_BATCH = (L // 128) // LC  # 4
N_TILES = BPC * TILES_PER_BATCH  # 32

_CACHE = {}


def _build_nc():
    import concourse.bacc as bacc
    import concourse.tile as tile
    import concourse.mybir as mybir

    f32 = mybir.dt.float32
    AF = mybir.ActivationFunctionType
    ALU = mybir.AluOpType

    nc = bacc.Bacc("TRN2", target_bir_lowering=False, debug=False)

    xs = nc.dram_tensor("xs", [BPC * L, D], f32, kind="ExternalInput")
    w1 = nc.dram_tensor("w1", [D, H], f32, kind="ExternalInput")
    w1t = nc.dram_tensor("w1t", [H, D], f32, kind="ExternalInput")
    w2 = nc.dram_tensor("w2", [H, 1], f32, kind="ExternalInput")
    b1 = nc.dram_tensor("b1", [H, 1], f32, kind="ExternalInput")
    b2 = nc.dram_tensor("b2", [1, 1], f32, kind="ExternalInput")
    noiset = nc.dram_tensor("noiset", [128, DC, BPC], f32, kind="ExternalInput")
    ws = nc.dram_tensor("ws", [1, BPC], f32, kind="ExternalOutput")

    # [t, p, lc, d] view of the x shard: partition p = row % 128.
    xview = xs.ap().rearrange("(t lc p) d -> t p lc d", lc=LC, p=128)

    with tile.TileContext(nc) as tc:
        with (
            tc.tile_pool(name="xin", bufs=6) as xin,
            tc.tile_pool(name="singles", bufs=1) as singles,
            tc.tile_pool(name="work", bufs=2) as work,
            tc.tile_pool(name="psA", bufs=2, space="PSUM") as psA,
            tc.tile_pool(name="psB", bufs=2, space="PSUM") as psB,
        ):
            # --- constants ---
            w1sb = singles.tile([128, DC, H], f32)
            nc.sync.dma_start(out=w1sb, in_=w1.ap().rearrange("(c p) h -> p c h", p=128))
            w1tsb = singles.tile([H, DC, 128], f32)
            nc.sync.dma_start(out=w1tsb, in_=w1t.ap().rearrange("h (c d) -> h c d", c=DC))
            w2sb = singles.tile([H, 1], f32)
            nc.sync.dma_start(out=w2sb, in_=w2.ap())
            b1sb = singles.tile([H, 1], f32)
            nc.sync.dma_start(out=b1sb, in_=b1.ap())
            b2sb = singles.tile([1, 1], f32)
            nc.sync.dma_start(out=b2sb, in_=b2.ap())
            noisesb = singles.tile([128, DC, BPC], f32)
            nc.sync.dma_start(out=noisesb, in_=noiset.ap())
            ones = singles.tile([128, 1], f32)
            nc.vector.memset(ones, 1.0)

            pooledT = singles.tile([128, DC, BPC], f32)
            xaT = singles.tile([128, DC, BPC], f32)

            # --- Phase A: pooled = mean_L(x), produced transposed ---
            psum_b = None
            for t in range(N_TILES):
                b = t // TILES_PER_BATCH
                tq = t % TILES_PER_BATCH
                xt = xin.tile([128, LC, D], f32)
                nc.sync.dma_start(out=xt, in_=xview[t])
                # fold the LC row-groups into group 0
                nc.vector.tensor_add(out=xt[:, 0, :], in0=xt[:, 0, :], in1=xt[:, 1, :])
                if tq == 0:
                    psum_b = psA.tile([128, DC], f32)
                for dc in range(DC):
                    # one accumulation group per psum_b zero region: start
                    # only on the very first matmul, stop on the very last
                    nc.tensor.matmul(
                        psum_b[:, dc : dc + 1],
                        xt[:, 0, dc * 128 : (dc + 1) * 128],
                        ones,
                        start=(tq == 0 and dc == 0),
                        stop=(tq == TILES_PER_BATCH - 1 and dc == DC - 1),
                    )
                if tq == TILES_PER_BATCH - 1:
                    # pooledT[:, :, b] = psum_b / L
                    nc.scalar.mul(out=pooledT[:, :, b], in_=psum_b, mul=1.0 / L)

            # x_adv0 = pooled + 0.01 * noise
            nc.vector.scalar_tensor_tensor(
                out=xaT, in0=noisesb, scalar=0.01, in1=pooledT,
                op0=ALU.mult, op1=ALU.add,
            )

            # --- Phase B: PGD ---
            for step in range(BUDGET):
                psz1 = psB.tile([H, BPC], f32)
                for dc in range(DC):
                    nc.tensor.matmul(
                        psz1,
                        w1sb[:, dc, :],
                        xaT[:, dc, :],
                        start=(dc == 0),
                        stop=(dc == DC - 1),
                    )
                # u = W2 * gelu'(z1 + b1)   (sigmoid' > 0 dropped: sign-invariant)
                u = work.tile([H, BPC], f32)
                nc.scalar.activation(
                    out=u, in_=psz1, func=AF.Derivative_Gelu, bias=b1sb, scale=1.0
                )
                nc.vector.tensor_scalar_mul(out=u, in0=u, scalar1=w2sb)

                psg = psB.tile([128, DC, BPC], f32)
                for dc in range(DC):
                    nc.tensor.matmul(
                        psg[:, dc, :], w1tsb[:, dc, :], u,
                        start=(dc == 0), stop=(dc == DC - 1),
                    )
                sgn = work.tile([128, DC, BPC], f32)
                nc.scalar.sign(out=sgn, in_=psg)
                # xa' = xa - STEP*sgn ; xa = pooled + clip(xa' - pooled, +-EPS)
                delta = work.tile([128, DC, BPC], f32)
                nc.vector.scalar_tensor_tensor(
                    out=delta, in0=sgn, scalar=-STEP, in1=xaT,
                    op0=ALU.mult, op1=ALU.add,
                )
                nc.vector.tensor_sub(out=delta, in0=delta, in1=pooledT)
                nc.vector.tensor_scalar(
                    out=delta, in0=delta, scalar1=-EPS, scalar2=EPS,
                    op0=ALU.max, op1=ALU.min,
                )
                nc.vector.tensor_add(out=xaT, in0=delta, in1=pooledT)

            # --- final score ---
            psz1 = psB.tile([H, BPC], f32)
            for dc in range(DC):
                nc.tensor.matmul(
                    psz1,
                    w1sb[:, dc, :],
                    xaT[:, dc, :],
                    start=(dc == 0),
                    stop=(dc == DC - 1),
                )
            hT = work.tile([H, BPC], f32)
            nc.scalar.activation(out=hT, in_=psz1, func=AF.Gelu, bias=b1sb, scale=1.0)
            psz2 = psB.tile([1, BPC], f32)
            nc.tensor.matmul(psz2, w2sb, hT, start=True, stop=True)
            s_sb = work.tile([1, BPC], f32)
            nc.scalar.activation(
                out=s_sb, in_=psz2, func=AF.Sigmoid, bias=b2sb, scale=1.0
            )
            nc.sync.dma_start(out=ws.ap(), in_=s_sb)

    nc.compile()
    return nc


def _get_nc():
    if "nc" not in _CACHE:
        _CACHE["nc"] = _build_nc()
    return _CACHE["nc"]


def _noise_host():
    """noise = jax.random.normal(key(1), (B, D), f32), computed on host CPU."""
    if "noise" not in _CACHE:
        import jax
        import jax.numpy as jnp

        cpu = jax.devices("cpu")[0]
        with jax.default_device(cpu):
            key = jax.random.key(1)
            _CACHE["noise"] = np.asarray(
                jax.random.normal(key, (B, D), dtype=jnp.float32)
            )
    return _CACHE["noise"]


def _in_maps(x, W1, b1, W2, b2):
    noise = _noise_host()
    w1 = np.ascontiguousarray(W1, dtype=np.float32)
    w1t = np.ascontiguousarray(W1.T, dtype=np.float32)
    w2 = np.ascontiguousarray(W2, dtype=np.float32).reshape(H, 1)
    b1r = np.ascontiguousarray(b1, dtype=np.float32).reshape(H, 1)
    b2r = np.ascontiguousarray(b2, dtype=np.float32).reshape(1, 1)
    maps = []
    for c in range(N_CORES):
        xs = np.ascontiguousarray(x[c * BPC : (c + 1) * BPC]).reshape(BPC * L, D)
        nslice = noise[c * BPC : (c + 1) * BPC]  # [8, 2048]
        noiset = np.ascontiguousarray(
            nslice.reshape(BPC, DC, 128).transpose(2, 1, 0)
        )  # [128, 16, 8]
        maps.append(
            {
                "xs": xs,
                "w1": w1,
                "w1t": w1t,
                "w2": w2,
                "b1": b1r,
                "b2": b2r,
                "noiset": noiset,
            }
        )
    return maps


def run_device(x, W1, b1, W2, b2):
    """Run the Bass kernel on 8 cores; returns worst_score [B] f32 and the
    raw BassKernelResults (for timing introspection)."""
    from concourse.bass_utils import run_bass_kernel_spmd

    nc = _get_nc()
    maps = _in_maps(x, W1, b1, W2, b2)
    res = run_bass_kernel_spmd(nc, maps, core_ids=list(range(N_CORES)))
    ws = np.concatenate([res.results[c]["ws"][0] for c in range(N_CORES)])
    return ws.astype(np.float32), res


def kernel(x, W1, b1, W2, b2):
    x = np.asarray(x)
    worst_score, _ = run_device(
        x,
        np.asarray(W1),
        np.asarray(b1),
        np.asarray(W2),
        np.asarray(b2),
    )
    cert_score = (np.float32(1.0) - worst_score).astype(np.float32)
    violated = np.bool_(worst_score.min() < np.float32(0.1))
    return x, worst_score, cert_score, violated


# revision 6
# speedup vs baseline: 1.0172x; 1.0172x over previous
"""Trainium2 Bass kernel for nn_CounterexampleGenerator (dense_mlp, memory-bound).

Strategy (8 NeuronCores, SPMD, no collectives):
  - Data-parallel over batch B=64: each core gets 8 batches = a contiguous
    [8192, 2048] f32 slice of x (64 MiB) — reading it once is the roofline.
  - Phase A (per core): stream x in 2 MiB tiles [128, 2, 2048]; one DVE add
    folds the two 128-row groups; PE column-sum matmuls (lhsT = tile chunk,
    rhs = ones) accumulate the L-reduction into PSUM, producing pooled in a
    TRANSPOSED layout pooledT[p, dc, b] = pooled[b, dc*128+p] (d on
    partitions) so the PGD loop needs no transposes at all.
  - Phase B: 10 PGD steps on [128, 16, 8] tiles. z1T = sum_dc W1c.T @ xaTc
    (W1 chunks stationary); u = W2 * gelu'(z1T + b1) via one ScalarE
    Derivative_Gelu (b1 is a per-partition bias in this layout) and one
    per-partition tensor_scalar_mul; gT chunks = W1Tc.T @ u (host-supplied
    W1.T chunks stationary); sign + clip updates on VectorE. The positive
    sigmoid' factor is dropped — it cannot change sign(grad).
  - Final score: Gelu, W2 matmul, Sigmoid; each core emits worst_score[1,8].
  - Host: gather 8x8 scores, cert = 1 - ws, violated = min(ws) < 0.1,
    x passes through untouched.
"""

import numpy as np

B, L, D, H = 64, 1024, 2048, 128
N_CORES = 8
BPC = B // N_CORES  # batches per core = 8
DC = D // 128  # 16 d-chunks
BUDGET = 10
STEP = 0.01
EPS = 0.2

# Phase-A tiling: each DMA tile holds LC row-groups of 128 rows x D cols.
LC = 2  # row-groups per tile -> [128, LC*2048] f32 = 2 MiB
TILES_PER_BATCH = (L // 128) // LC  # 4
N_TILES = BPC * TILES_PER_BATCH  # 32

BF16_PGD = False  # bf16 weights/acts for PGD matmuls (FWL: faster LDWEIGHTS)

_CACHE = {}


def _setup(nc, tile, mybir):
    """Declare DRAM tensors and return handles dict."""
    f32 = mybir.dt.float32
    xs = nc.dram_tensor("xs", [BPC * L, D], f32, kind="ExternalInput")
    w1 = nc.dram_tensor("w1", [D, H], f32, kind="ExternalInput")
    w2 = nc.dram_tensor("w2", [H, 1], f32, kind="ExternalInput")
    b1 = nc.dram_tensor("b1", [H, 1], f32, kind="ExternalInput")
    b2 = nc.dram_tensor("b2", [1, 1], f32, kind="ExternalInput")
    noiset = nc.dram_tensor("noiset", [128, DC, BPC], f32, kind="ExternalInput")
    ws = nc.dram_tensor("ws", [1, BPC], f32, kind="ExternalOutput")
    return dict(xs=xs, w1=w1, w2=w2, b1=b1, b2=b2, noiset=noiset, ws=ws)


def _load_consts(nc, mybir, h, singles, psB):
    """Load weights/constants into SBUF (once); W1^T built on-chip via PE."""
    from concourse.masks import make_identity

    f32 = mybir.dt.float32
    bf16 = mybir.dt.bfloat16
    wdt = bf16 if BF16_PGD else f32
    w1sb = singles.tile([128, DC, H], f32)
    nc.sync.dma_start(out=w1sb, in_=h["w1"].ap().rearrange("(c p) h -> p c h", p=128))
    identity = singles.tile([128, 128], f32)
    make_identity(nc, identity)
    w1tsb = singles.tile([H, DC, 128], wdt)
    for dc in range(DC):
        pst = psB.tile([128, 128], f32, tag="psg")
        nc.tensor.transpose(pst, w1sb[:, dc, :], identity)
        nc.scalar.copy(out=w1tsb[:, dc, :], in_=pst)
    w2sb = singles.tile([H, 1], f32)
    nc.sync.dma_start(out=w2sb, in_=h["w2"].ap())
    b1sb = singles.tile([H, 1], f32)
    nc.sync.dma_start(out=b1sb, in_=h["b1"].ap())
    b2sb = singles.tile([1, 1], f32)
    nc.sync.dma_start(out=b2sb, in_=h["b2"].ap())
    noisesb = singles.tile([128, DC, BPC], f32)
    nc.sync.dma_start(out=noisesb, in_=h["noiset"].ap())
    ones = singles.tile([128, 1], f32)
    nc.vector.memset(ones, 1.0)
    pooledT = singles.tile([128, DC, BPC], f32)
    xaT = singles.tile([128, DC, BPC], f32)
    if BF16_PGD:
        w1sb_m = singles.tile([128, DC, H], bf16)
        nc.vector.tensor_copy(out=w1sb_m, in_=w1sb)
    else:
        w1sb_m = w1sb
    deltaT = singles.tile([128, DC, BPC], f32)
    return dict(w1sb=w1sb, w1sb_m=w1sb_m, w1tsb=w1tsb, w2sb=w2sb, b1sb=b1sb,
                b2sb=b2sb, noisesb=noisesb, ones=ones, pooledT=pooledT, xaT=xaT,
                deltaT=deltaT)


def _emit_body(nc, mybir, h, c, xin, work, psA, psB):
    """Emit one full pass: phase A + PGD + final score + output DMA."""
    f32 = mybir.dt.float32
    AF = mybir.ActivationFunctionType
    ALU = mybir.AluOpType
    xview = h["xs"].ap().rearrange("(t lc p) d -> t p lc d", lc=LC, p=128)
    w1sb, w1tsb, w2sb, b1sb, b2sb = (
        c["w1sb"], c["w1tsb"], c["w2sb"], c["b1sb"], c["b2sb"])
    noisesb, ones, pooledT, xaT = c["noisesb"], c["ones"], c["pooledT"], c["xaT"]
    if True:
        if True:
            # --- Phase A: pooled = mean_L(x), produced transposed ---
            psum_b = None
            for t in range(N_TILES):
                b = t // TILES_PER_BATCH
                tq = t % TILES_PER_BATCH
                xt = xin.tile([128, LC, D], f32)
                nc.sync.dma_start(out=xt, in_=xview[t])
                # fold the LC row-groups into group 0
                nc.vector.tensor_add(out=xt[:, 0, :], in0=xt[:, 0, :], in1=xt[:, 1, :])
                if tq == 0:
                    psum_b = psA.tile([128, DC], f32)
                for dc in range(DC):
                    # one accumulation group per psum_b zero region: start
                    # only on the very first matmul, stop on the very last
                    nc.tensor.matmul(
                        psum_b[:, dc : dc + 1],
                        xt[:, 0, dc * 128 : (dc + 1) * 128],
                        ones,
                        start=(tq == 0 and dc == 0),
                        stop=(tq == TILES_PER_BATCH - 1 and dc == DC - 1),
                    )
                if tq == TILES_PER_BATCH - 1:
                    # pooledT[:, :, b] = psum_b / L
                    nc.scalar.mul(out=pooledT[:, :, b], in_=psum_b, mul=1.0 / L)

            # delta0 = 0.01 * noise ; xa0 = pooled + delta0
            deltaT = c["deltaT"]
            w1sb_m = c["w1sb_m"]
            bf16 = mybir.dt.bfloat16
            nc.vector.tensor_scalar_mul(out=deltaT, in0=noisesb, scalar1=0.01)
            nc.vector.tensor_add(out=xaT, in0=pooledT, in1=deltaT)

            # --- Phase B: PGD (state: deltaT = xa - pooled, and xaT) ---
            for step in range(BUDGET):
                if BF16_PGD:
                    xa_mm = work.tile([128, DC, BPC], bf16, tag="xabf")
                    nc.vector.tensor_copy(out=xa_mm, in_=xaT)
                else:
                    xa_mm = xaT
                psz1 = psB.tile([H, BPC], f32)
                for dc in range(DC):
                    nc.tensor.matmul(
                        psz1,
                        w1sb_m[:, dc, :],
                        xa_mm[:, dc, :],
                        start=(dc == 0),
                        stop=(dc == DC - 1),
                    )
                # u = W2 * gelu'(z1 + b1)   (sigmoid' > 0 dropped: sign-invariant)
                # both ops on ScalarE: no DVE hop in the chain
                u = work.tile([H, BPC], f32)
                nc.scalar.activation(
                    out=u, in_=psz1, func=AF.Derivative_Gelu, bias=b1sb, scale=1.0
                )
                u_mm = work.tile([H, BPC], bf16 if BF16_PGD else f32, tag="umm")
                nc.scalar.mul(out=u_mm, in_=u, mul=w2sb)

                psg = psB.tile([128, DC, BPC], f32)
                for dc in range(DC):
                    nc.tensor.matmul(
                        psg[:, dc, :], w1tsb[:, dc, :], u_mm,
                        start=(dc == 0), stop=(dc == DC - 1),
                    )
                sgn = work.tile([128, DC, BPC], f32)
                nc.scalar.sign(out=sgn, in_=psg)
                # delta = clip(delta - STEP*sgn, +-EPS) ; xa = pooled + delta
                nc.vector.scalar_tensor_tensor(
                    out=deltaT, in0=sgn, scalar=-STEP, in1=deltaT,
                    op0=ALU.mult, op1=ALU.add,
                )
                nc.vector.tensor_scalar(
                    out=deltaT, in0=deltaT, scalar1=-EPS, scalar2=EPS,
                    op0=ALU.max, op1=ALU.min,
                )
                nc.vector.tensor_add(out=xaT, in0=pooledT, in1=deltaT)

            # --- final score ---
            psz1 = psB.tile([H, BPC], f32)
            for dc in range(DC):
                nc.tensor.matmul(
                    psz1,
                    w1sb[:, dc, :],
                    xaT[:, dc, :],
                    start=(dc == 0),
                    stop=(dc == DC - 1),
                )
            hT = work.tile([H, BPC], f32)
            nc.scalar.activation(out=hT, in_=psz1, func=AF.Gelu, bias=b1sb, scale=1.0)
            psz2 = psB.tile([1, BPC], f32)
            nc.tensor.matmul(psz2, w2sb, hT, start=True, stop=True)
            s_sb = work.tile([1, BPC], f32)
            nc.scalar.activation(
                out=s_sb, in_=psz2, func=AF.Sigmoid, bias=b2sb, scale=1.0
            )
            nc.sync.dma_start(out=h["ws"].ap(), in_=s_sb)


def _build_nc():
    import concourse.bacc as bacc
    import concourse.tile as tile
    import concourse.mybir as mybir

    nc = bacc.Bacc("TRN2", target_bir_lowering=False, debug=False)
    h = _setup(nc, tile, mybir)
    with tile.TileContext(nc) as tc:
        with (
            tc.tile_pool(name="xin", bufs=6) as xin,
            tc.tile_pool(name="singles", bufs=1) as singles,
            tc.tile_pool(name="work", bufs=2) as work,
            tc.tile_pool(name="psA", bufs=2, space="PSUM") as psA,
            tc.tile_pool(name="psB", bufs=2, space="PSUM") as psB,
        ):
            c = _load_consts(nc, mybir, h, singles, psB)
            _emit_body(nc, mybir, h, c, xin, work, psA, psB)
    nc.compile()
    return nc


def _get_nc():
    if "nc" not in _CACHE:
        _CACHE["nc"] = _build_nc()
    return _CACHE["nc"]


def _noise_host():
    """noise = jax.random.normal(key(1), (B, D), f32), computed on host CPU."""
    if "noise" not in _CACHE:
        import jax
        import jax.numpy as jnp

        cpu = jax.devices("cpu")[0]
        with jax.default_device(cpu):
            key = jax.random.key(1)
            _CACHE["noise"] = np.asarray(
                jax.random.normal(key, (B, D), dtype=jnp.float32)
            )
    return _CACHE["noise"]


def _in_maps(x, W1, b1, W2, b2):
    noise = _noise_host()
    w1 = np.ascontiguousarray(W1, dtype=np.float32)
    w2 = np.ascontiguousarray(W2, dtype=np.float32).reshape(H, 1)
    b1r = np.ascontiguousarray(b1, dtype=np.float32).reshape(H, 1)
    b2r = np.ascontiguousarray(b2, dtype=np.float32).reshape(1, 1)
    maps = []
    for c in range(N_CORES):
        xs = np.ascontiguousarray(x[c * BPC : (c + 1) * BPC]).reshape(BPC * L, D)
        nslice = noise[c * BPC : (c + 1) * BPC]  # [8, 2048]
        noiset = np.ascontiguousarray(
            nslice.reshape(BPC, DC, 128).transpose(2, 1, 0)
        )  # [128, 16, 8]
        maps.append(
            {
                "xs": xs,
                "w1": w1,
                "w2": w2,
                "b1": b1r,
                "b2": b2r,
                "noiset": noiset,
            }
        )
    return maps


def run_device(x, W1, b1, W2, b2):
    """Run the Bass kernel on 8 cores; returns worst_score [B] f32 and the
    raw BassKernelResults (for timing introspection)."""
    from concourse.bass_utils import run_bass_kernel_spmd

    nc = _get_nc()
    maps = _in_maps(x, W1, b1, W2, b2)
    res = run_bass_kernel_spmd(nc, maps, core_ids=list(range(N_CORES)))
    ws = np.concatenate([res.results[c]["ws"][0] for c in range(N_CORES)])
    return ws.astype(np.float32), res


def kernel(x, W1, b1, W2, b2):
    x = np.asarray(x)
    worst_score, _ = run_device(
        x,
        np.asarray(W1),
        np.asarray(b1),
        np.asarray(W2),
        np.asarray(b2),
    )
    cert_score = (np.float32(1.0) - worst_score).astype(np.float32)
    violated = np.bool_(worst_score.min() < np.float32(0.1))
    return x, worst_score, cert_score, violated


# revision 9
# speedup vs baseline: 1.0302x; 1.0129x over previous
"""Trainium2 Bass kernel for nn_CounterexampleGenerator (dense_mlp, memory-bound).

Strategy (8 NeuronCores, SPMD, no collectives):
  - Data-parallel over batch B=64: each core gets 8 batches = a contiguous
    [8192, 2048] f32 slice of x (64 MiB) — reading it once is the roofline.
  - Phase A (per core): stream x in 2 MiB tiles [128, 2, 2048]; one DVE add
    folds the two 128-row groups; PE column-sum matmuls (lhsT = tile chunk,
    rhs = ones) accumulate the L-reduction into PSUM, producing pooled in a
    TRANSPOSED layout pooledT[p, dc, b] = pooled[b, dc*128+p] (d on
    partitions) so the PGD loop needs no transposes at all.
  - Phase B: 10 PGD steps on [128, 16, 8] tiles. z1T = sum_dc W1c.T @ xaTc
    (W1 chunks stationary); u = W2 * gelu'(z1T + b1) via one ScalarE
    Derivative_Gelu (b1 is a per-partition bias in this layout) and one
    per-partition tensor_scalar_mul; gT chunks = W1Tc.T @ u (host-supplied
    W1.T chunks stationary); sign + clip updates on VectorE. The positive
    sigmoid' factor is dropped — it cannot change sign(grad).
  - Final score: Gelu, W2 matmul, Sigmoid; each core emits worst_score[1,8].
  - Host: gather 8x8 scores, cert = 1 - ws, violated = min(ws) < 0.1,
    x passes through untouched.
"""

import numpy as np

B, L, D, H = 64, 1024, 2048, 128
N_CORES = 8
BPC = B // N_CORES  # batches per core = 8
DC = D // 128  # 16 d-chunks
BUDGET = 10
STEP = 0.01
EPS = 0.2

# Phase-A tiling: each DMA tile holds LC row-groups of 128 rows x D cols.
LC = 2  # row-groups per tile -> [128, LC*2048] f32 = 2 MiB
TILES_PER_BATCH = (L // 128) // LC  # 4
N_TILES = BPC * TILES_PER_BATCH  # 32

BF16_PGD = False  # bf16 weights/acts for PGD matmuls (FWL: faster LDWEIGHTS)

_CACHE = {}


def _setup(nc, tile, mybir):
    """Declare DRAM tensors and return handles dict."""
    f32 = mybir.dt.float32
    xs = nc.dram_tensor("xs", [BPC * L, D], f32, kind="ExternalInput")
    w1 = nc.dram_tensor("w1", [D, H], f32, kind="ExternalInput")
    w2 = nc.dram_tensor("w2", [H, 1], f32, kind="ExternalInput")
    b1 = nc.dram_tensor("b1", [H, 1], f32, kind="ExternalInput")
    b2 = nc.dram_tensor("b2", [1, 1], f32, kind="ExternalInput")
    noiset = nc.dram_tensor("noiset", [128, DC, BPC], f32, kind="ExternalInput")
    ws = nc.dram_tensor("ws", [1, BPC], f32, kind="ExternalOutput")
    return dict(xs=xs, w1=w1, w2=w2, b1=b1, b2=b2, noiset=noiset, ws=ws)


def _load_consts(nc, mybir, h, singles, psB):
    """Load weights/constants into SBUF (once); W1^T built on-chip via PE."""
    from concourse.masks import make_identity

    f32 = mybir.dt.float32
    bf16 = mybir.dt.bfloat16
    wdt = bf16 if BF16_PGD else f32
    w1sb = singles.tile([128, DC, H], f32)
    nc.sync.dma_start(out=w1sb, in_=h["w1"].ap().rearrange("(c p) h -> p c h", p=128))
    identity = singles.tile([128, 128], f32)
    make_identity(nc, identity)
    w1tsb = singles.tile([H, DC, 128], wdt)
    for dc in range(DC):
        pst = psB.tile([128, 128], f32, tag="psg")
        nc.tensor.transpose(pst, w1sb[:, dc, :], identity)
        nc.scalar.copy(out=w1tsb[:, dc, :], in_=pst)
    w2sb = singles.tile([H, 1], f32)
    nc.sync.dma_start(out=w2sb, in_=h["w2"].ap())
    b1sb = singles.tile([H, 1], f32)
    nc.sync.dma_start(out=b1sb, in_=h["b1"].ap())
    b2sb = singles.tile([1, 1], f32)
    nc.sync.dma_start(out=b2sb, in_=h["b2"].ap())
    noisesb = singles.tile([128, DC, BPC], f32)
    nc.sync.dma_start(out=noisesb, in_=h["noiset"].ap())
    ones = singles.tile([128, 1], f32)
    nc.vector.memset(ones, 1.0)
    # touch Derivative_Gelu early so its ACT table load (~1.3us) hides under
    # the phase-A DMA stream instead of stalling the first PGD step
    warm = singles.tile([1, 1], f32)
    nc.scalar.activation(
        out=warm, in_=ones[:1, :],
        func=mybir.ActivationFunctionType.Derivative_Gelu,
    )
    pooledT = singles.tile([128, DC, BPC], f32)
    xaT = singles.tile([128, DC, BPC], f32)
    if BF16_PGD:
        w1sb_m = singles.tile([128, DC, H], bf16)
        nc.vector.tensor_copy(out=w1sb_m, in_=w1sb)
    else:
        w1sb_m = w1sb
    deltaT = singles.tile([128, DC, BPC], f32)
    return dict(w1sb=w1sb, w1sb_m=w1sb_m, w1tsb=w1tsb, w2sb=w2sb, b1sb=b1sb,
                b2sb=b2sb, noisesb=noisesb, ones=ones, pooledT=pooledT, xaT=xaT,
                deltaT=deltaT)


def _emit_body(nc, mybir, h, c, xin, work, psA, psB):
    """Emit one full pass: phase A + PGD + final score + output DMA."""
    f32 = mybir.dt.float32
    AF = mybir.ActivationFunctionType
    ALU = mybir.AluOpType
    xview = h["xs"].ap().rearrange("(t lc p) d -> t p lc d", lc=LC, p=128)
    w1sb, w1tsb, w2sb, b1sb, b2sb = (
        c["w1sb"], c["w1tsb"], c["w2sb"], c["b1sb"], c["b2sb"])
    noisesb, ones, pooledT, xaT = c["noisesb"], c["ones"], c["pooledT"], c["xaT"]
    if True:
        if True:
            # --- Phase A: pooled = mean_L(x), produced transposed ---
            psum_b = None
            for t in range(N_TILES):
                b = t // TILES_PER_BATCH
                tq = t % TILES_PER_BATCH
                xt = xin.tile([128, LC, D], f32)
                nc.sync.dma_start(out=xt, in_=xview[t])
                # fold the LC row-groups into group 0
                nc.vector.tensor_add(out=xt[:, 0, :], in0=xt[:, 0, :], in1=xt[:, 1, :])
                if tq == 0:
                    psum_b = psA.tile([128, DC], f32)
                for dc in range(DC):
                    # one accumulation group per psum_b zero region: start
                    # only on the very first matmul, stop on the very last
                    nc.tensor.matmul(
                        psum_b[:, dc : dc + 1],
                        xt[:, 0, dc * 128 : (dc + 1) * 128],
                        ones,
                        start=(tq == 0 and dc == 0),
                        stop=(tq == TILES_PER_BATCH - 1 and dc == DC - 1),
                    )
                if tq == TILES_PER_BATCH - 1:
                    # pooledT[:, :, b] = psum_b / L
                    nc.scalar.mul(out=pooledT[:, :, b], in_=psum_b, mul=1.0 / L)

            # delta0 = 0.01 * noise ; xa0 = pooled + delta0
            deltaT = c["deltaT"]
            w1sb_m = c["w1sb_m"]
            bf16 = mybir.dt.bfloat16
            nc.vector.tensor_scalar_mul(out=deltaT, in0=noisesb, scalar1=0.01)
            nc.vector.tensor_add(out=xaT, in0=pooledT, in1=deltaT)

            # --- Phase B: PGD (state: deltaT = xa - pooled, and xaT) ---
            for step in range(BUDGET):
                if BF16_PGD:
                    xa_mm = work.tile([128, DC, BPC], bf16, tag="xabf")
                    nc.vector.tensor_copy(out=xa_mm, in_=xaT)
                else:
                    xa_mm = xaT
                psz1 = psB.tile([H, BPC], f32)
                for dc in range(DC):
                    nc.tensor.matmul(
                        psz1,
                        w1sb_m[:, dc, :],
                        xa_mm[:, dc, :],
                        start=(dc == 0),
                        stop=(dc == DC - 1),
                    )
                # u = W2 * gelu'(z1 + b1)   (sigmoid' > 0 dropped: sign-invariant)
                # both ops on ScalarE: no DVE hop in the chain
                u = work.tile([H, BPC], f32)
                nc.scalar.activation(
                    out=u, in_=psz1, func=AF.Derivative_Gelu, bias=b1sb, scale=1.0
                )
                u_mm = work.tile([H, BPC], bf16 if BF16_PGD else f32, tag="umm")
                nc.scalar.mul(out=u_mm, in_=u, mul=w2sb)

                psg = psB.tile([128, DC, BPC], f32)
                for dc in range(DC):
                    nc.tensor.matmul(
                        psg[:, dc, :], w1tsb[:, dc, :], u_mm,
                        start=(dc == 0), stop=(dc == DC - 1),
                    )
                # m = (g > 0) - 0.5 in {-0.5, +0.5}; delta -= 2*STEP*m
                # (== delta - STEP*sign(g); sign(g)==0 has measure zero)
                sgn = work.tile([128, DC, BPC], f32)
                nc.vector.tensor_scalar(
                    out=sgn, in0=psg, scalar1=0.0, scalar2=0.5,
                    op0=ALU.is_gt, op1=ALU.subtract,
                )
                nc.vector.scalar_tensor_tensor(
                    out=deltaT, in0=sgn, scalar=-2.0 * STEP, in1=deltaT,
                    op0=ALU.mult, op1=ALU.add,
                )
                nc.vector.tensor_scalar(
                    out=deltaT, in0=deltaT, scalar1=-EPS, scalar2=EPS,
                    op0=ALU.max, op1=ALU.min,
                )
                nc.vector.tensor_add(out=xaT, in0=pooledT, in1=deltaT)

            # --- final score ---
            psz1 = psB.tile([H, BPC], f32)
            for dc in range(DC):
                nc.tensor.matmul(
                    psz1,
                    w1sb[:, dc, :],
                    xaT[:, dc, :],
                    start=(dc == 0),
                    stop=(dc == DC - 1),
                )
            hT = work.tile([H, BPC], f32)
            nc.scalar.activation(out=hT, in_=psz1, func=AF.Gelu, bias=b1sb, scale=1.0)
            psz2 = psB.tile([1, BPC], f32)
            nc.tensor.matmul(psz2, w2sb, hT, start=True, stop=True)
            # sigmoid (+ b2) applied on host; ship raw z2
            s_sb = work.tile([1, BPC], f32)
            nc.vector.tensor_copy(out=s_sb, in_=psz2)
            nc.sync.dma_start(out=h["ws"].ap(), in_=s_sb)


def _build_nc():
    import concourse.bacc as bacc
    import concourse.tile as tile
    import concourse.mybir as mybir

    nc = bacc.Bacc("TRN2", target_bir_lowering=False, debug=False)
    h = _setup(nc, tile, mybir)
    with tile.TileContext(nc) as tc:
        with (
            tc.tile_pool(name="xin", bufs=6) as xin,
            tc.tile_pool(name="singles", bufs=1) as singles,
            tc.tile_pool(name="work", bufs=2) as work,
            tc.tile_pool(name="psA", bufs=2, space="PSUM") as psA,
            tc.tile_pool(name="psB", bufs=2, space="PSUM") as psB,
        ):
            c = _load_consts(nc, mybir, h, singles, psB)
            _emit_body(nc, mybir, h, c, xin, work, psA, psB)
    nc.compile()
    return nc


def _get_nc():
    if "nc" not in _CACHE:
        _CACHE["nc"] = _build_nc()
    return _CACHE["nc"]


def _noise_host():
    """noise = jax.random.normal(key(1), (B, D), f32), computed on host CPU."""
    if "noise" not in _CACHE:
        import jax
        import jax.numpy as jnp

        cpu = jax.devices("cpu")[0]
        with jax.default_device(cpu):
            key = jax.random.key(1)
            _CACHE["noise"] = np.asarray(
                jax.random.normal(key, (B, D), dtype=jnp.float32)
            )
    return _CACHE["noise"]


def _in_maps(x, W1, b1, W2, b2):
    noise = _noise_host()
    w1 = np.ascontiguousarray(W1, dtype=np.float32)
    w2 = np.ascontiguousarray(W2, dtype=np.float32).reshape(H, 1)
    b1r = np.ascontiguousarray(b1, dtype=np.float32).reshape(H, 1)
    b2r = np.ascontiguousarray(b2, dtype=np.float32).reshape(1, 1)
    maps = []
    for c in range(N_CORES):
        xs = np.ascontiguousarray(x[c * BPC : (c + 1) * BPC]).reshape(BPC * L, D)
        nslice = noise[c * BPC : (c + 1) * BPC]  # [8, 2048]
        noiset = np.ascontiguousarray(
            nslice.reshape(BPC, DC, 128).transpose(2, 1, 0)
        )  # [128, 16, 8]
        maps.append(
            {
                "xs": xs,
                "w1": w1,
                "w2": w2,
                "b1": b1r,
                "b2": b2r,
                "noiset": noiset,
            }
        )
    return maps


def run_device(x, W1, b1, W2, b2):
    """Run the Bass kernel on 8 cores; returns worst_score [B] f32 and the
    raw BassKernelResults (for timing introspection)."""
    from concourse.bass_utils import run_bass_kernel_spmd

    nc = _get_nc()
    maps = _in_maps(x, W1, b1, W2, b2)
    res = None
    last_exc = None
    for _attempt in range(3):
        try:
            res = run_bass_kernel_spmd(nc, maps, core_ids=list(range(N_CORES)))
            break
        except Exception as e:  # rare transient device/runtime hiccups
            last_exc = e
            import time as _time

            _time.sleep(2.0)
    if res is None:
        raise last_exc
    z2 = np.concatenate([res.results[c]["ws"][0] for c in range(N_CORES)])
    z2 = z2.astype(np.float32) + np.asarray(b2, np.float32).reshape(-1)[0]
    ws = (np.float32(1.0) / (np.float32(1.0) + np.exp(-z2, dtype=np.float32)))
    return ws.astype(np.float32), res


def kernel(x, W1, b1, W2, b2):
    x = np.asarray(x)
    worst_score, _ = run_device(
        x,
        np.asarray(W1),
        np.asarray(b1),
        np.asarray(W2),
        np.asarray(b2),
    )
    cert_score = (np.float32(1.0) - worst_score).astype(np.float32)
    violated = np.bool_(worst_score.min() < np.float32(0.1))
    return x, worst_score, cert_score, violated
